# revision 2
# baseline (speedup 1.0000x reference)
"""Hard-triplet miner for Trainium2, 8-core SPMD.

Per core (strip of 1024 anchors):

Negatives: the PE computes the masked score matrix directly,
    w[i, j] = G~[i, j] - 2*[lab_i == lab_j]
as ONE psum accumulation per 512-column chunk built from 5 bf16 matmuls:
3 split-precision matmuls for the Gram part (x = xh + xl in bf16;
G~ = xh.xh + xh.xl + xl.xh, error ~1e-7) and 2 one-hot label matmuls
(K=256 one-hot contraction; 0/±2 values are exact in bf16).  Since
d = sqrt(2-2G) is monotone decreasing in G, hardest negative =
argmax_j w.  The DVE then does only TWO big ops per 128-row tile:
a row max (the needle) and one max_index pass (the position).

Positives: same-label candidates only (~32 per row), mined on a tiny
padded block-diagonal problem: labels sorted into 64-row slots, 16
[128, 128] tiles per core; a CPU-precomputed +BIG additive mask kills
the diagonal / other-label / padding entries, then one segmented
reduce-min + two max_index calls extract all 16 tiles' answers.

keep and all index remapping are label-only math, done on the CPU.
"""

import numpy as np
import ml_dtypes

import concourse.bacc as bacc
import concourse.bass as bass
import concourse.mybir as mybir
import concourse.tile as tile
from concourse.bass_utils import run_bass_kernel_spmd

F32 = mybir.dt.float32
BF16 = mybir.dt.bfloat16
U32 = mybir.dt.uint32
BF = ml_dtypes.bfloat16

N = 8192          # total rows
D = 128           # embed dim
C = 256           # num labels
NCORES = 8
STRIP = N // NCORES       # 1024 anchor rows per core
RT = STRIP // 128         # 8 row-tiles per core
CT = N // 512             # 16 column chunks of 512
SLOT = 64                 # padded rows per label (max label count 49 < 64)
NPAD = C * SLOT           # 16384 padded positive rows
PPC = NPAD // NCORES      # 2048 padded rows per core
PT = PPC // 128           # 16 positive tiles per core
PAD_VAL = 3.0e38
BIG = 1.0e30


def build_program(k_repeat: int = 1, use_for_i: bool = False):
    nc = bacc.Bacc("TRN2", target_bir_lowering=False, debug=False,
                   num_devices=NCORES)

    # shared (same array on every core)
    xhT_d = nc.dram_tensor("xhT", [D, N], BF16, kind="ExternalInput")
    xlT_d = nc.dram_tensor("xlT", [D, N], BF16, kind="ExternalInput")
    mk0_d = nc.dram_tensor("mk0T", [128, N], BF16, kind="ExternalInput")
    mk1_d = nc.dram_tensor("mk1T", [128, N], BF16, kind="ExternalInput")
    # per-core
    xhs_d = nc.dram_tensor("xh_sT", [D, STRIP], BF16, kind="ExternalInput")
    xls_d = nc.dram_tensor("xl_sT", [D, STRIP], BF16, kind="ExternalInput")
    oh0_d = nc.dram_tensor("oh0", [128, STRIP], BF16, kind="ExternalInput")
    oh1_d = nc.dram_tensor("oh1", [128, STRIP], BF16, kind="ExternalInput")
    xph_d = nc.dram_tensor("xphT", [D, PPC], BF16, kind="ExternalInput")
    xpl_d = nc.dram_tensor("xplT", [D, PPC], BF16, kind="ExternalInput")
    pm_d = nc.dram_tensor("pmask", [128, PPC], F32, kind="ExternalInput")
    # outputs
    neg_d = nc.dram_tensor("neg_out", [128, 8 * RT], U32, kind="ExternalOutput")
    pos_d = nc.dram_tensor("pos_out", [128, PT], U32, kind="ExternalOutput")

    with tile.TileContext(nc) as tc:
        with (
            tc.tile_pool(name="persist", bufs=1) as persist,
            tc.tile_pool(name="wpool", bufs=2) as wpool,
            tc.tile_pool(name="psA", bufs=6, space=bass.MemorySpace.PSUM) as psA,
            tc.tile_pool(name="psP", bufs=2, space=bass.MemorySpace.PSUM) as psP,
        ):
            xhT = persist.tile([D, N], BF16, tag="xhT")
            xlT = persist.tile([D, N], BF16, tag="xlT")
            mk0 = persist.tile([128, N], BF16, tag="mk0")
            mk1 = persist.tile([128, N], BF16, tag="mk1")
            xhs = persist.tile([D, STRIP], BF16, tag="xhs")
            xls = persist.tile([D, STRIP], BF16, tag="xls")
            oh0 = persist.tile([128, STRIP], BF16, tag="oh0")
            oh1 = persist.tile([128, STRIP], BF16, tag="oh1")
            xph = persist.tile([D, PPC], BF16, tag="xph")
            xpl = persist.tile([D, PPC], BF16, tag="xpl")
            pm = persist.tile([128, PPC], F32, tag="pm")
            for t, d in ((xhT, xhT_d), (xlT, xlT_d), (mk0, mk0_d),
                         (mk1, mk1_d), (xhs, xhs_d), (xls, xls_d),
                         (oh0, oh0_d), (oh1, oh1_d), (xph, xph_d),
                         (xpl, xpl_d), (pm, pm_d)):
                nc.sync.dma_start(t[:], d[:])

            wpos = persist.tile([128, PPC], F32, tag="wpos")
            wq = persist.tile([128, PPC], F32, tag="wq")
            m16 = persist.tile([128, PT], F32, tag="m16")
            # needle slots: [:, rt*8] = row max of w_rt; slots 1-7 stay PAD
            inm8 = persist.tile([128, 8 * RT], F32, tag="inm8")
            nc.vector.memset(inm8[:], PAD_VAL)
            neg_idx = persist.tile([128, 8 * RT], U32, tag="neg_idx")
            pos_idx = persist.tile([128, PT], U32, tag="pos_idx")

            def body():
                # ---- positives (tiny padded block-diagonal problem) ----
                for t in range(PT):
                    sl = slice(t * 128, (t + 1) * 128)
                    ps = psP.tile([128, 128], F32, tag="psP")
                    nc.tensor.matmul(ps[:], xph[:, sl], xph[:, sl],
                                     start=True, stop=False)
                    nc.tensor.matmul(ps[:], xph[:, sl], xpl[:, sl],
                                     start=False, stop=False)
                    nc.tensor.matmul(ps[:], xpl[:, sl], xph[:, sl],
                                     start=False, stop=True)
                    nc.scalar.activation(wpos[:, sl], ps[:],
                                         mybir.ActivationFunctionType.Copy)
                nc.vector.tensor_tensor(wq[:], wpos[:], pm[:],
                                        mybir.AluOpType.add)
                nc.vector.tensor_reduce(
                    m16[:], wq[:].rearrange("p (t k) -> p t k", t=PT),
                    mybir.AxisListType.X, mybir.AluOpType.min)
                nc.vector.max_index(pos_idx[:, 0:8], m16[:, 0:8],
                                    wq[:, 0:1024])
                nc.vector.max_index(pos_idx[:, 8:16], m16[:, 8:16],
                                    wq[:, 1024:2048])

                # ---- negatives (full-width strips) ----
                for rt in range(RT):
                    rsl = slice(rt * 128, (rt + 1) * 128)
                    w = wpool.tile([128, N], F32, tag="w")
                    for ct in range(CT):
                        csl = slice(ct * 512, (ct + 1) * 512)
                        ps = psA.tile([128, 512], F32, tag="psA")
                        nc.tensor.matmul(ps[:], xhs[:, rsl], xhT[:, csl],
                                         start=True, stop=False)
                        nc.tensor.matmul(ps[:], xhs[:, rsl], xlT[:, csl],
                                         start=False, stop=False)
                        nc.tensor.matmul(ps[:], xls[:, rsl], xhT[:, csl],
                                         start=False, stop=False)
                        nc.tensor.matmul(ps[:], oh0[:, rsl], mk0[:, csl],
                                         start=False, stop=False)
                        nc.tensor.matmul(ps[:], oh1[:, rsl], mk1[:, csl],
                                         start=False, stop=True)
                        nc.scalar.activation(
                            w[:, csl], ps[:],
                            mybir.ActivationFunctionType.Copy)
                    nc.vector.tensor_reduce(inm8[:, rt * 8:rt * 8 + 1], w[:],
                                            mybir.AxisListType.X,
                                            mybir.AluOpType.max)
                    nc.vector.max_index(neg_idx[:, rt * 8:(rt + 1) * 8],
                                        inm8[:, rt * 8:(rt + 1) * 8], w[:])

            if use_for_i:
                with tc.For_i(0, k_repeat, 1,
                              hint_engines=(mybir.EngineType.PE,)):
                    body()
            else:
                for _ in range(k_repeat):
                    body()

            nc.sync.dma_start(neg_d[:], neg_idx[:])
            nc.sync.dma_start(pos_d[:], pos_idx[:])

    nc.compile()
    return nc


def prepare(l_embeds: np.ndarray, l_labels: np.ndarray):
    """CPU-side prep: normalization, bf16 hi/lo split, one-hots, padded
    positive layout.  Returns (in_maps, aux) where aux carries the index
    remapping arrays."""
    x = np.asarray(l_embeds, dtype=np.float32)
    lab = np.asarray(l_labels).astype(np.int64)
    xn = x / np.linalg.norm(x, axis=1, keepdims=True)

    xh32 = xn.astype(BF).astype(np.float32)
    xh = xh32.astype(BF)
    xl = (xn - xh32).astype(BF)
    xhT = np.ascontiguousarray(xh.T)           # [128, N]
    xlT = np.ascontiguousarray(xl.T)

    cc = np.arange(128)
    mk0 = (-2.0 * (lab[None, :] == cc[:, None])).astype(BF)      # [128, N]
    mk1 = (-2.0 * (lab[None, :] == (cc + 128)[:, None])).astype(BF)

    # padded positive layout: label c occupies rows [c*SLOT, c*SLOT+n_c)
    counts = np.bincount(lab, minlength=C)
    if counts.max() > SLOT:
        raise ValueError(f"label count {counts.max()} exceeds SLOT={SLOT}")
    order = np.argsort(lab, kind="stable")
    within = np.arange(N) - np.cumsum(np.concatenate([[0], counts]))[lab[order]]
    slot_of = lab[order] * SLOT + within       # padded row for order[i]
    padded2orig = np.full(NPAD, -1, dtype=np.int64)
    padded2orig[slot_of] = order
    xp = np.zeros((NPAD, D), dtype=np.float32)
    xp[slot_of] = xn[order]
    xph32 = xp.astype(BF).astype(np.float32)
    xph = xph32.astype(BF)
    xpl = (xp - xph32).astype(BF)

    # positive mask: +BIG everywhere except (same 64-block, not self, real)
    real = padded2orig >= 0                    # [NPAD]
    p_ar = np.arange(128)
    in_maps = []
    for m in range(NCORES):
        ssl = slice(m * STRIP, (m + 1) * STRIP)
        psl = slice(m * PPC, (m + 1) * PPC)
        lab_s = lab[ssl]
        oh0 = (lab_s[None, :] == cc[:, None]).astype(BF)
        oh1 = (lab_s[None, :] == (cc + 128)[:, None]).astype(BF)

        pmask = np.full((128, PPC), BIG, dtype=np.float32)
        base = m * PPC
        for t in range(PT):
            rows = base + t * 128 + p_ar               # padded row ids
            cols = base + t * 128 + p_ar               # candidate ids
            same_block = (rows[:, None] // SLOT) == (cols[None, :] // SLOT)
            ok = (same_block & (p_ar[:, None] != p_ar[None, :])
                  & real[rows][:, None] & real[cols][None, :])
            pmask[:, t * 128:(t + 1) * 128] = np.where(ok, 0.0, BIG)

        in_maps.append({
            "xhT": xhT, "xlT": xlT, "mk0T": mk0, "mk1T": mk1,
            "xh_sT": np.ascontiguousarray(xhT[:, ssl]),
            "xl_sT": np.ascontiguousarray(xlT[:, ssl]),
            "oh0": oh0, "oh1": oh1,
            "xphT": np.ascontiguousarray(xph[psl].T),
            "xplT": np.ascontiguousarray(xpl[psl].T),
            "pmask": pmask,
        })

    idt = np.int32 if np.asarray(l_labels).dtype != np.int64 else np.int64
    aux = {"padded2orig": padded2orig, "counts": counts, "lab": lab,
           "idt": idt}
    return in_maps, aux


def postprocess(results, aux):
    padded2orig = aux["padded2orig"]
    counts = aux["counts"]
    lab = aux["lab"]
    idt = aux["idt"]

    neg = np.empty(N, np.int64)
    pos = np.empty(N, np.int64)
    for m in range(NCORES):
        r = results[m]
        # negatives: row p of row-tile rt = anchor m*1024 + rt*128 + p
        ni = r["neg_out"].astype(np.int64)      # [128, 64]
        for rt in range(RT):
            neg[m * STRIP + rt * 128:m * STRIP + (rt + 1) * 128] = \
                ni[:, rt * 8]
        # positives: padded row m*PPC + t*128 + p; answer is a position in
        # the 1024-wide half -> padded candidate id -> original row
        pi = r["pos_out"].astype(np.int64)      # [128, 16]
        for t in range(PT):
            q = pi[:, t]
            cand_pad = m * PPC + (t // 8) * 1024 + q
            cand_pad = np.where((q >= 0) & (q < 2048), cand_pad, 0)
            orig_row = padded2orig[m * PPC + t * 128 + np.arange(128)]
            cand = padded2orig[cand_pad]
            sel = orig_row >= 0
            pos[orig_row[sel]] = np.maximum(cand[sel], 0)

    anchor = np.arange(N, dtype=idt)
    keep = counts[lab] >= 2                    # a real positive exists
    keep &= counts[lab] <= N - 1               # a negative exists
    return (anchor, pos.astype(idt), neg.astype(idt), keep)


_CACHED_NC = None


def kernel(l_embeds: np.ndarray, l_labels: np.ndarray):
    global _CACHED_NC
    if _CACHED_NC is None:
        _CACHED_NC = build_program()
    nc = _CACHED_NC
    in_maps, aux = prepare(l_embeds, l_labels)
    res = run_bass_kernel_spmd(nc, in_maps, list(range(NCORES))).results
    return postprocess(res, aux)


# revision 15
# speedup vs baseline: 6.4096x; 6.4096x over previous
"""Hard-triplet miner for Trainium2, 8-core SPMD.

Per core (strip of 1024 anchors):

Negatives: the PE computes the masked score matrix directly,
    w[i, j] = G~[i, j] - 2*[lab_i == lab_j]
as ONE psum accumulation per 512-column chunk built from 5 bf16 matmuls:
3 split-precision matmuls for the Gram part (x = xh + xl in bf16;
G~ = xh.xh + xh.xl + xl.xh, error ~1e-7) and 2 one-hot label matmuls
(K=256 one-hot contraction; 0/±2 values are exact in bf16).  Since
d = sqrt(2-2G) is monotone decreasing in G, hardest negative =
argmax_j w.  The DVE then does only TWO big ops per 128-row tile:
a row max (the needle) and one max_index pass (the position).

Positives: same-label candidates only (~32 per row), mined on a tiny
padded block-diagonal problem: labels sorted into 64-row slots, 16
[128, 128] tiles per core; a CPU-precomputed +BIG additive mask kills
the diagonal / other-label / padding entries, then one segmented
reduce-min + two max_index calls extract all 16 tiles' answers.

keep and all index remapping are label-only math, done on the CPU.
"""

import numpy as np
import ml_dtypes

import concourse.bacc as bacc
import concourse.bass as bass
import concourse.mybir as mybir
import concourse.tile as tile
from concourse import masks
from concourse.bass_utils import run_bass_kernel_spmd

F32 = mybir.dt.float32
BF16 = mybir.dt.bfloat16
U32 = mybir.dt.uint32
BF = ml_dtypes.bfloat16

N = 8192          # total rows
D = 128           # embed dim
C = 256           # num labels
NCORES = 8
STRIP = N // NCORES       # 1024 anchor rows per core
RT = STRIP // 128         # 8 row-tiles per core
CT = N // 512             # 16 column chunks of 512
SLOT = 64                 # padded rows per label (max label count 49 < 64)
NPAD = C * SLOT           # 16384 padded positive rows
PPC = NPAD // NCORES      # 2048 padded rows per core
PT = PPC // 128           # 16 positive tiles per core
PAD_VAL = 3.0e38
BIG = 1.0e30


def build_program(k_repeat: int = 1, use_for_i: bool = False, dbg: int = 0):
    nc = bacc.Bacc("TRN2", target_bir_lowering=False, debug=False,
                   num_devices=NCORES)

    # shared (same array on every core)
    xhT_d = nc.dram_tensor("xhT", [D, N], BF16, kind="ExternalInput")
    xlT_d = nc.dram_tensor("xlT", [D, N], BF16, kind="ExternalInput")

    # per-core
    xhs_d = nc.dram_tensor("xh_sT", [D, STRIP], BF16, kind="ExternalInput")
    xls_d = nc.dram_tensor("xl_sT", [D, STRIP], BF16, kind="ExternalInput")
    mstr_d = nc.dram_tensor("maskstr", [128, RT * N], BF16,
                            kind="ExternalInput")
    xph_d = nc.dram_tensor("xphT", [D, PPC], BF16, kind="ExternalInput")
    xpl_d = nc.dram_tensor("xplT", [D, PPC], BF16, kind="ExternalInput")
    pm_d = nc.dram_tensor("pmask", [128, PPC], BF16, kind="ExternalInput")
    # outputs
    neg_d = nc.dram_tensor("neg_out", [128, 8 * RT], U32, kind="ExternalOutput")
    pos_d = nc.dram_tensor("pos_out", [128, PT], U32, kind="ExternalOutput")

    with tile.TileContext(nc) as tc:
        with (
            tc.tile_pool(name="persist", bufs=1) as persist,
            tc.tile_pool(name="wpool", bufs=3) as wpool,
            tc.tile_pool(name="mstage", bufs=2) as mstage,
            tc.tile_pool(name="psA", bufs=2, space=bass.MemorySpace.PSUM) as psA,
        ):
            xhT = persist.tile([D, N], BF16, tag="xhT")
            xlT = persist.tile([D, N], BF16, tag="xlT")
            xhs = persist.tile([D, STRIP], BF16, tag="xhs")
            xls = persist.tile([D, STRIP], BF16, tag="xls")
            xph = persist.tile([D, PPC], BF16, tag="xph")
            xpl = persist.tile([D, PPC], BF16, tag="xpl")
            pm = persist.tile([128, PPC], BF16, tag="pm")
            identb = persist.tile([128, 128], BF16, tag="identb")
            for t, d in ((xhT, xhT_d), (xlT, xlT_d),
                         (xhs, xhs_d), (xls, xls_d), (xph, xph_d),
                         (xpl, xpl_d), (pm, pm_d)):
                nc.sync.dma_start(t[:], d[:])
            identf = persist.tile([128, 128], F32, tag="identf")
            masks.make_identity(nc, identf[:])
            nc.vector.tensor_copy(identb[:], identf[:])

            wq = persist.tile([128, PPC], F32, tag="wq")
            if dbg == 9:
                wq_big = persist.tile([128, N], F32, tag="wq_big")
                nc.vector.memset(wq_big[:], 0.0)
            if dbg == 11:
                mfix = persist.tile([128, N], BF16, tag="mfix")
                nc.vector.memset(mfix[:], 0.0)
            m16 = persist.tile([128, PT], F32, tag="m16")
            # needle slots: [:, rt*8] = row max of w_rt; slots 1-7 stay PAD
            inm8 = persist.tile([128, 8 * RT], F32, tag="inm8")
            nc.vector.memset(inm8[:], PAD_VAL)
            neg_idx = persist.tile([128, 8 * RT], U32, tag="neg_idx")
            nc.vector.memset(neg_idx[:], 0)
            pos_idx = persist.tile([128, PT], U32, tag="pos_idx")
            nc.vector.memset(pos_idx[:], 0)
            nc.vector.memset(wq[:], 0.0)

            def body():
                if dbg == 10:
                    for rt in range(RT):
                        mst = mstage.tile([128, N], BF16, tag="mst")
                        nc.sync.dma_start(mst[:],
                                          mstr_d[:, rt * N:(rt + 1) * N])
                        nc.vector.tensor_copy(inm8[:, rt:rt + 1],
                                              mst[:, 0:2].bitcast(F32))
                    return
                if dbg == 9:
                    # pure-DVE loop: isolate DVE throughput in this structure
                    for rt in range(RT):
                        nc.vector.tensor_reduce(inm8[:, rt * 8:rt * 8 + 1],
                                                wq_big[:],
                                                mybir.AxisListType.X,
                                                mybir.AluOpType.max)
                        nc.vector.max_index(neg_idx[:, rt * 8:(rt + 1) * 8],
                                            inm8[:, rt * 8:(rt + 1) * 8],
                                            wq_big[:])
                    return
                # ---- negatives (full-width strips) ----
                for rt in range(RT if dbg != 6 else 0):
                    rsl = slice(rt * 128, (rt + 1) * 128)
                    w = wpool.tile([128, N], F32, tag="w")
                    if dbg != 11:
                        mst = mstage.tile([128, N], BF16, tag="mst")
                        nc.sync.dma_start(mst[:],
                                          mstr_d[:, rt * N:(rt + 1) * N])
                    else:
                        mst = mfix
                    for pt_ in range(N // 2048):
                        ps = psA.tile([128, 2048], F32, tag="psA")
                        for sub in range(4):
                            ct = pt_ * 4 + sub
                            csl = slice(ct * 512, (ct + 1) * 512)
                            sl2 = slice(sub * 512, (sub + 1) * 512)
                            if dbg != 5:
                                nc.tensor.matmul(ps[:, sl2], identb[:],
                                                 mst[:, csl],
                                                 start=True, stop=False)
                            nc.tensor.matmul(ps[:, sl2], xhs[:, rsl],
                                             xhT[:, csl],
                                             start=(dbg == 5), stop=False)
                            nc.tensor.matmul(ps[:, sl2], xhs[:, rsl],
                                             xlT[:, csl],
                                             start=False, stop=False)
                            nc.tensor.matmul(ps[:, sl2], xls[:, rsl],
                                             xhT[:, csl],
                                             start=False, stop=True)
                        if dbg not in (4, 5):
                            nc.scalar.activation(
                                w[:, pt_ * 2048:(pt_ + 1) * 2048], ps[:],
                                mybir.ActivationFunctionType.Copy)
                    if dbg not in (3, 4, 5):
                        nc.vector.tensor_reduce(inm8[:, rt * 8:rt * 8 + 1],
                                                w[:], mybir.AxisListType.X,
                                                mybir.AluOpType.max)
                        nc.vector.max_index(neg_idx[:, rt * 8:(rt + 1) * 8],
                                            inm8[:, rt * 8:(rt + 1) * 8],
                                            w[:])

                # ---- positives (tiny padded block-diagonal problem) ----
                for t in range(PT if dbg not in (2, 4, 5) else 0):
                    sl = slice(t * 128, (t + 1) * 128)
                    ps = psA.tile([128, 2048], F32, tag="psA")
                    nc.tensor.matmul(ps[:, 0:128], xph[:, sl], xph[:, sl],
                                     start=True, stop=False)
                    nc.tensor.matmul(ps[:, 0:128], xph[:, sl], xpl[:, sl],
                                     start=False, stop=False)
                    nc.tensor.matmul(ps[:, 0:128], xpl[:, sl], xph[:, sl],
                                     start=False, stop=False)
                    nc.tensor.matmul(ps[:, 0:128], identb[:], pm[:, sl],
                                     start=False, stop=True)
                    nc.scalar.activation(wq[:, sl], ps[:, 0:128],
                                         mybir.ActivationFunctionType.Copy)
                if dbg not in (2, 4, 5):
                    nc.vector.tensor_reduce(
                        m16[:], wq[:].rearrange("p (t k) -> p t k", t=PT),
                        mybir.AxisListType.X, mybir.AluOpType.min)
                    nc.vector.max_index(pos_idx[:, 0:8], m16[:, 0:8],
                                        wq[:, 0:1024])
                    nc.vector.max_index(pos_idx[:, 8:16], m16[:, 8:16],
                                        wq[:, 1024:2048])

            if use_for_i:
                with tc.For_i(0, k_repeat, 1,
                              hint_engines=(mybir.EngineType.PE,),
                              staggered_reset=True):
                    body()
            else:
                for _ in range(k_repeat):
                    body()

            nc.sync.dma_start(neg_d[:], neg_idx[:])
            nc.sync.dma_start(pos_d[:], pos_idx[:])

    nc.compile()
    return nc


def prepare(l_embeds: np.ndarray, l_labels: np.ndarray):
    """CPU-side prep: normalization, bf16 hi/lo split, one-hots, padded
    positive layout.  Returns (in_maps, aux) where aux carries the index
    remapping arrays."""
    x = np.asarray(l_embeds, dtype=np.float32)
    lab = np.asarray(l_labels).astype(np.int64)
    xn = x / np.linalg.norm(x, axis=1, keepdims=True)

    xh32 = xn.astype(BF).astype(np.float32)
    xh = xh32.astype(BF)
    xl = (xn - xh32).astype(BF)
    xhT = np.ascontiguousarray(xh.T)           # [128, N]
    xlT = np.ascontiguousarray(xl.T)


    # padded positive layout: label c occupies rows [c*SLOT, c*SLOT+n_c)
    counts = np.bincount(lab, minlength=C)
    pos_on_cpu = counts.max() > SLOT   # never for the reference data (max 49)
    order = np.argsort(lab, kind="stable")
    within = np.arange(N) - np.cumsum(np.concatenate([[0], counts]))[lab[order]]
    slot_of = lab[order] * SLOT + within       # padded row for order[i]
    padded2orig = np.full(NPAD, -1, dtype=np.int64)
    padded2orig[slot_of] = order
    xp = np.zeros((NPAD, D), dtype=np.float32)
    xp[slot_of] = xn[order]
    xph32 = xp.astype(BF).astype(np.float32)
    xph = xph32.astype(BF)
    xpl = (xp - xph32).astype(BF)

    # positive mask: +BIG everywhere except (same 64-block, not self, real)
    real = padded2orig >= 0                    # [NPAD]
    p_ar = np.arange(128)
    in_maps = []
    for m in range(NCORES):
        ssl = slice(m * STRIP, (m + 1) * STRIP)
        psl = slice(m * PPC, (m + 1) * PPC)
        lab_s = lab[ssl]
        # mask stream [128, RT*N]: [p, rt*N + j] = -2*[lab_s[rt*128+p]==lab_j]
        eq = (lab_s[:, None] == lab[None, :])            # [1024, N]
        mstr = np.where(eq, np.float32(-2.0), np.float32(0.0))
        mstr = np.ascontiguousarray(
            mstr.reshape(RT, 128, N).transpose(1, 0, 2).reshape(128, RT * N)
        ).astype(BF)

        pmask = np.full((128, PPC), BIG, dtype=np.float32)
        base = m * PPC
        for t in range(PT):
            rows = base + t * 128 + p_ar               # padded row ids
            cols = base + t * 128 + p_ar               # candidate ids
            same_block = (rows[:, None] // SLOT) == (cols[None, :] // SLOT)
            ok = (same_block & (p_ar[:, None] != p_ar[None, :])
                  & real[rows][:, None] & real[cols][None, :])
            pmask[:, t * 128:(t + 1) * 128] = np.where(ok, 0.0, BIG)

        in_maps.append({
            "xhT": xhT, "xlT": xlT, "maskstr": mstr,
            "xh_sT": np.ascontiguousarray(xhT[:, ssl]),
            "xl_sT": np.ascontiguousarray(xlT[:, ssl]),
            "xphT": np.ascontiguousarray(xph[psl].T),
            "xplT": np.ascontiguousarray(xpl[psl].T),
            "pmask": pmask.astype(BF),
        })

    idt = np.int32 if np.asarray(l_labels).dtype != np.int64 else np.int64
    aux = {"padded2orig": padded2orig, "counts": counts, "lab": lab,
           "idt": idt, "pos_on_cpu": pos_on_cpu, "xn": xn}
    return in_maps, aux


def _pos_cpu(xn, lab):
    """Fallback hardest-positive mining on CPU (only if a label overflows
    SLOT, which the reference data never does)."""
    pos = np.zeros(N, np.int64)
    for c in np.unique(lab):
        rows = np.nonzero(lab == c)[0]
        if len(rows) < 2:
            continue
        g = xn[rows] @ xn[rows].T
        np.fill_diagonal(g, np.inf)
        pos[rows] = rows[np.argmin(g, axis=1)]
    return pos


def postprocess(results, aux):
    padded2orig = aux["padded2orig"]
    counts = aux["counts"]
    lab = aux["lab"]
    idt = aux["idt"]

    neg = np.empty(N, np.int64)
    pos = np.empty(N, np.int64)
    for m in range(NCORES):
        r = results[m]
        # negatives: row p of row-tile rt = anchor m*1024 + rt*128 + p
        ni = r["neg_out"].astype(np.int64)      # [128, 64]
        for rt in range(RT):
            neg[m * STRIP + rt * 128:m * STRIP + (rt + 1) * 128] = \
                ni[:, rt * 8]
        # positives: padded row m*PPC + t*128 + p; answer is a position in
        # the 1024-wide half -> padded candidate id -> original row
        pi = r["pos_out"].astype(np.int64)      # [128, 16]
        for t in range(PT):
            q = pi[:, t]
            cand_pad = m * PPC + (t // 8) * 1024 + q
            cand_pad = np.where((q >= 0) & (q < 2048), cand_pad, 0)
            orig_row = padded2orig[m * PPC + t * 128 + np.arange(128)]
            cand = padded2orig[cand_pad]
            sel = orig_row >= 0
            pos[orig_row[sel]] = np.maximum(cand[sel], 0)

    if aux["pos_on_cpu"]:
        pos = _pos_cpu(aux["xn"], lab)
    anchor = np.arange(N, dtype=idt)
    keep = counts[lab] >= 2                    # a real positive exists
    keep &= counts[lab] <= N - 1               # a negative exists
    return (anchor, pos.astype(idt), neg.astype(idt), keep)


_CACHED_NC = None


def kernel(l_embeds: np.ndarray, l_labels: np.ndarray):
    global _CACHED_NC
    if _CACHED_NC is None:
        _CACHED_NC = build_program()
    nc = _CACHED_NC
    in_maps, aux = prepare(l_embeds, l_labels)
    res = run_bass_kernel_spmd(nc, in_maps, list(range(NCORES))).results
    return postprocess(res, aux)


# revision 16
# speedup vs baseline: 9.4751x; 1.4783x over previous
"""Hard-triplet miner for Trainium2, 8-core SPMD.

Per core (strip of 1024 anchors):

Negatives: the PE computes the masked score matrix directly,
    w[i, j] = G~[i, j] - 2*[lab_i == lab_j]
as ONE psum accumulation per 512-column chunk built from 4 bf16 matmuls:
an identity-weight matmul that injects a DMA-streamed, CPU-precomputed
bf16 mask chunk (-2 at same-label entries; DMA runs on otherwise-idle
engines, ~2MB per row-tile), plus 3 split-precision Gram matmuls
(x = xh + xl in bf16; G~ = xh.xh + xh.xl + xl.xh, error ~1e-7 vs fp32).
Since d = sqrt(2-2G) is monotone decreasing in G, hardest negative =
argmax_j w.  The DVE then does only TWO big ops per 128-row tile
(its 1x-rate streaming floor for exact value+index extraction):
a row max (the needle) and one max_index pass (the position).

Positives: same-label candidates only (~32 per row), mined on a tiny
padded block-diagonal problem: labels sorted into 64-row slots, 16
[128, 128] tiles per core; a CPU-precomputed +BIG mask (injected via
the same identity-matmul trick) kills the diagonal / other-label /
padding entries, then one segmented reduce-min + two 8-needle
max_index calls extract all 16 tiles' answers.

keep and all index remapping are label-only math, done on the CPU.
"""

import numpy as np
import ml_dtypes

import concourse.bacc as bacc
import concourse.bass as bass
import concourse.mybir as mybir
import concourse.tile as tile
from concourse import masks
from concourse.bass_utils import run_bass_kernel_spmd

F32 = mybir.dt.float32
BF16 = mybir.dt.bfloat16
U32 = mybir.dt.uint32
BF = ml_dtypes.bfloat16

N = 8192          # total rows
D = 128           # embed dim
C = 256           # num labels
NCORES = 8
STRIP = N // NCORES       # 1024 anchor rows per core
RT = STRIP // 128         # 8 row-tiles per core
CT = N // 512             # 16 column chunks of 512
SLOT = 64                 # padded rows per label (max label count 49 < 64)
NPAD = C * SLOT           # 16384 padded positive rows
PPC = NPAD // NCORES      # 2048 padded rows per core
PT = PPC // 128           # 16 positive tiles per core
PAD_VAL = 3.0e38
BIG = 1.0e30


def build_program(k_repeat: int = 1, use_for_i: bool = False, dbg: int = 0):
    nc = bacc.Bacc("TRN2", target_bir_lowering=False, debug=False,
                   num_devices=NCORES)

    # shared (same array on every core)
    xhT_d = nc.dram_tensor("xhT", [D, N], BF16, kind="ExternalInput")
    xlT_d = nc.dram_tensor("xlT", [D, N], BF16, kind="ExternalInput")

    # per-core
    xhs_d = nc.dram_tensor("xh_sT", [D, STRIP], BF16, kind="ExternalInput")
    xls_d = nc.dram_tensor("xl_sT", [D, STRIP], BF16, kind="ExternalInput")
    mstr_d = nc.dram_tensor("maskstr", [128, RT * N], BF16,
                            kind="ExternalInput")
    xph_d = nc.dram_tensor("xphT", [D, PPC], BF16, kind="ExternalInput")
    xpl_d = nc.dram_tensor("xplT", [D, PPC], BF16, kind="ExternalInput")
    pm_d = nc.dram_tensor("pmask", [128, PPC], BF16, kind="ExternalInput")
    # outputs
    neg_d = nc.dram_tensor("neg_out", [128, 8 * RT], U32, kind="ExternalOutput")
    pos_d = nc.dram_tensor("pos_out", [128, PT], U32, kind="ExternalOutput")

    with tile.TileContext(nc) as tc:
        with (
            tc.tile_pool(name="persist", bufs=1) as persist,
            tc.tile_pool(name="wpool", bufs=3) as wpool,
            tc.tile_pool(name="mstage", bufs=2) as mstage,
            tc.tile_pool(name="psA", bufs=2, space=bass.MemorySpace.PSUM) as psA,
        ):
            xhT = persist.tile([D, N], BF16, tag="xhT")
            xlT = persist.tile([D, N], BF16, tag="xlT")
            xhs = persist.tile([D, STRIP], BF16, tag="xhs")
            xls = persist.tile([D, STRIP], BF16, tag="xls")
            xph = persist.tile([D, PPC], BF16, tag="xph")
            xpl = persist.tile([D, PPC], BF16, tag="xpl")
            pm = persist.tile([128, PPC], BF16, tag="pm")
            identb = persist.tile([128, 128], BF16, tag="identb")
            for t, d in ((xhT, xhT_d), (xlT, xlT_d),
                         (xhs, xhs_d), (xls, xls_d), (xph, xph_d),
                         (xpl, xpl_d), (pm, pm_d)):
                nc.sync.dma_start(t[:], d[:])
            identf = persist.tile([128, 128], F32, tag="identf")
            masks.make_identity(nc, identf[:])
            nc.vector.tensor_copy(identb[:], identf[:])

            wq = persist.tile([128, PPC], F32, tag="wq")
            if dbg == 9:
                wq_big = persist.tile([128, N], F32, tag="wq_big")
                nc.vector.memset(wq_big[:], 0.0)
            if dbg == 11:
                mfix = persist.tile([128, N], BF16, tag="mfix")
                nc.vector.memset(mfix[:], 0.0)
            m16 = persist.tile([128, PT], F32, tag="m16")
            # needle slots: [:, rt*8] = row max of w_rt; slots 1-7 stay PAD
            inm8 = persist.tile([128, 8 * RT], F32, tag="inm8")
            nc.vector.memset(inm8[:], PAD_VAL)
            neg_idx = persist.tile([128, 8 * RT], U32, tag="neg_idx")
            nc.vector.memset(neg_idx[:], 0)
            pos_idx = persist.tile([128, PT], U32, tag="pos_idx")
            nc.vector.memset(pos_idx[:], 0)
            nc.vector.memset(wq[:], 0.0)

            def body():
                if dbg == 10:
                    for rt in range(RT):
                        mst = mstage.tile([128, N], BF16, tag="mst")
                        nc.sync.dma_start(mst[:],
                                          mstr_d[:, rt * N:(rt + 1) * N])
                        nc.vector.tensor_copy(inm8[:, rt:rt + 1],
                                              mst[:, 0:2].bitcast(F32))
                    return
                if dbg == 9:
                    # pure-DVE loop: isolate DVE throughput in this structure
                    for rt in range(RT):
                        nc.vector.tensor_reduce(inm8[:, rt * 8:rt * 8 + 1],
                                                wq_big[:],
                                                mybir.AxisListType.X,
                                                mybir.AluOpType.max)
                        nc.vector.max_index(neg_idx[:, rt * 8:(rt + 1) * 8],
                                            inm8[:, rt * 8:(rt + 1) * 8],
                                            wq_big[:])
                    return
                # ---- negatives (full-width strips) ----
                for rt in range(RT if dbg != 6 else 0):
                    rsl = slice(rt * 128, (rt + 1) * 128)
                    w = wpool.tile([128, N], F32, tag="w")
                    if dbg != 11:
                        mst = mstage.tile([128, N], BF16, tag="mst")
                        nc.sync.dma_start(mst[:],
                                          mstr_d[:, rt * N:(rt + 1) * N])
                    else:
                        mst = mfix
                    for pt_ in range(N // 2048):
                        ps = psA.tile([128, 2048], F32, tag="psA")
                        for sub in range(4):
                            ct = pt_ * 4 + sub
                            csl = slice(ct * 512, (ct + 1) * 512)
                            sl2 = slice(sub * 512, (sub + 1) * 512)
                            if dbg != 5:
                                nc.tensor.matmul(ps[:, sl2], identb[:],
                                                 mst[:, csl],
                                                 start=True, stop=False)
                            nc.tensor.matmul(ps[:, sl2], xhs[:, rsl],
                                             xhT[:, csl],
                                             start=(dbg == 5), stop=False)
                            nc.tensor.matmul(ps[:, sl2], xhs[:, rsl],
                                             xlT[:, csl],
                                             start=False, stop=False)
                            nc.tensor.matmul(ps[:, sl2], xls[:, rsl],
                                             xhT[:, csl],
                                             start=False, stop=True)
                        if dbg not in (4, 5):
                            nc.scalar.activation(
                                w[:, pt_ * 2048:(pt_ + 1) * 2048], ps[:],
                                mybir.ActivationFunctionType.Copy)
                    if dbg not in (3, 4, 5):
                        nc.vector.tensor_reduce(inm8[:, rt * 8:rt * 8 + 1],
                                                w[:], mybir.AxisListType.X,
                                                mybir.AluOpType.max)
                        nc.vector.max_index(neg_idx[:, rt * 8:(rt + 1) * 8],
                                            inm8[:, rt * 8:(rt + 1) * 8],
                                            w[:])

                # ---- positives (tiny padded block-diagonal problem) ----
                for t in range(PT if dbg not in (2, 4, 5) else 0):
                    sl = slice(t * 128, (t + 1) * 128)
                    ps = psA.tile([128, 2048], F32, tag="psA")
                    nc.tensor.matmul(ps[:, 0:128], xph[:, sl], xph[:, sl],
                                     start=True, stop=False)
                    nc.tensor.matmul(ps[:, 0:128], xph[:, sl], xpl[:, sl],
                                     start=False, stop=False)
                    nc.tensor.matmul(ps[:, 0:128], xpl[:, sl], xph[:, sl],
                                     start=False, stop=False)
                    nc.tensor.matmul(ps[:, 0:128], identb[:], pm[:, sl],
                                     start=False, stop=True)
                    nc.scalar.activation(wq[:, sl], ps[:, 0:128],
                                         mybir.ActivationFunctionType.Copy)
                if dbg not in (2, 4, 5):
                    nc.vector.tensor_reduce(
                        m16[:], wq[:].rearrange("p (t k) -> p t k", t=PT),
                        mybir.AxisListType.X, mybir.AluOpType.min)
                    nc.vector.max_index(pos_idx[:, 0:8], m16[:, 0:8],
                                        wq[:, 0:1024])
                    nc.vector.max_index(pos_idx[:, 8:16], m16[:, 8:16],
                                        wq[:, 1024:2048])

            if use_for_i:
                with tc.For_i(0, k_repeat, 1,
                              hint_engines=(mybir.EngineType.PE,),
                              staggered_reset=True):
                    body()
            else:
                for _ in range(k_repeat):
                    body()

            nc.sync.dma_start(neg_d[:], neg_idx[:])
            nc.sync.dma_start(pos_d[:], pos_idx[:])

    nc.compile()
    return nc


def prepare(l_embeds: np.ndarray, l_labels: np.ndarray):
    """CPU-side prep: normalization, bf16 hi/lo split, one-hots, padded
    positive layout.  Returns (in_maps, aux) where aux carries the index
    remapping arrays."""
    x = np.asarray(l_embeds, dtype=np.float32)
    lab = np.asarray(l_labels).astype(np.int64)
    xn = x / np.linalg.norm(x, axis=1, keepdims=True)

    xh32 = xn.astype(BF).astype(np.float32)
    xh = xh32.astype(BF)
    xl = (xn - xh32).astype(BF)
    xhT = np.ascontiguousarray(xh.T)           # [128, N]
    xlT = np.ascontiguousarray(xl.T)


    # padded positive layout: label c occupies rows [c*SLOT, c*SLOT+n_c)
    counts = np.bincount(lab, minlength=C)
    pos_on_cpu = counts.max() > SLOT   # never for the reference data (max 49)
    order = np.argsort(lab, kind="stable")
    within = np.arange(N) - np.cumsum(np.concatenate([[0], counts]))[lab[order]]
    slot_of = lab[order] * SLOT + within       # padded row for order[i]
    padded2orig = np.full(NPAD, -1, dtype=np.int64)
    padded2orig[slot_of] = order
    xp = np.zeros((NPAD, D), dtype=np.float32)
    xp[slot_of] = xn[order]
    xph32 = xp.astype(BF).astype(np.float32)
    xph = xph32.astype(BF)
    xpl = (xp - xph32).astype(BF)

    # positive mask: +BIG everywhere except (same 64-block, not self, real)
    real = padded2orig >= 0                    # [NPAD]
    p_ar = np.arange(128)
    in_maps = []
    for m in range(NCORES):
        ssl = slice(m * STRIP, (m + 1) * STRIP)
        psl = slice(m * PPC, (m + 1) * PPC)
        lab_s = lab[ssl]
        # mask stream [128, RT*N]: [p, rt*N + j] = -2*[lab_s[rt*128+p]==lab_j]
        eq = (lab_s[:, None] == lab[None, :])            # [1024, N]
        mstr = np.where(eq, np.float32(-2.0), np.float32(0.0))
        mstr = np.ascontiguousarray(
            mstr.reshape(RT, 128, N).transpose(1, 0, 2).reshape(128, RT * N)
        ).astype(BF)

        pmask = np.full((128, PPC), BIG, dtype=np.float32)
        base = m * PPC
        for t in range(PT):
            rows = base + t * 128 + p_ar               # padded row ids
            cols = base + t * 128 + p_ar               # candidate ids
            same_block = (rows[:, None] // SLOT) == (cols[None, :] // SLOT)
            ok = (same_block & (p_ar[:, None] != p_ar[None, :])
                  & real[rows][:, None] & real[cols][None, :])
            pmask[:, t * 128:(t + 1) * 128] = np.where(ok, 0.0, BIG)

        in_maps.append({
            "xhT": xhT, "xlT": xlT, "maskstr": mstr,
            "xh_sT": np.ascontiguousarray(xhT[:, ssl]),
            "xl_sT": np.ascontiguousarray(xlT[:, ssl]),
            "xphT": np.ascontiguousarray(xph[psl].T),
            "xplT": np.ascontiguousarray(xpl[psl].T),
            "pmask": pmask.astype(BF),
        })

    idt = np.int32 if np.asarray(l_labels).dtype != np.int64 else np.int64
    aux = {"padded2orig": padded2orig, "counts": counts, "lab": lab,
           "idt": idt, "pos_on_cpu": pos_on_cpu, "xn": xn}
    return in_maps, aux


def _pos_cpu(xn, lab):
    """Fallback hardest-positive mining on CPU (only if a label overflows
    SLOT, which the reference data never does)."""
    pos = np.zeros(N, np.int64)
    for c in np.unique(lab):
        rows = np.nonzero(lab == c)[0]
        if len(rows) < 2:
            continue
        g = xn[rows] @ xn[rows].T
        np.fill_diagonal(g, np.inf)
        pos[rows] = rows[np.argmin(g, axis=1)]
    return pos


def postprocess(results, aux):
    padded2orig = aux["padded2orig"]
    counts = aux["counts"]
    lab = aux["lab"]
    idt = aux["idt"]

    neg = np.empty(N, np.int64)
    pos = np.empty(N, np.int64)
    for m in range(NCORES):
        r = results[m]
        # negatives: row p of row-tile rt = anchor m*1024 + rt*128 + p
        ni = r["neg_out"].astype(np.int64)      # [128, 64]
        for rt in range(RT):
            neg[m * STRIP + rt * 128:m * STRIP + (rt + 1) * 128] = \
                ni[:, rt * 8]
        # positives: padded row m*PPC + t*128 + p; answer is a position in
        # the 1024-wide half -> padded candidate id -> original row
        pi = r["pos_out"].astype(np.int64)      # [128, 16]
        for t in range(PT):
            q = pi[:, t]
            cand_pad = m * PPC + (t // 8) * 1024 + q
            cand_pad = np.where((q >= 0) & (q < 2048), cand_pad, 0)
            orig_row = padded2orig[m * PPC + t * 128 + np.arange(128)]
            cand = padded2orig[cand_pad]
            sel = orig_row >= 0
            pos[orig_row[sel]] = np.maximum(cand[sel], 0)

    if aux["pos_on_cpu"]:
        pos = _pos_cpu(aux["xn"], lab)
    anchor = np.arange(N, dtype=idt)
    keep = counts[lab] >= 2                    # a real positive exists
    keep &= counts[lab] <= N - 1               # a negative exists
    return (anchor, pos.astype(idt), neg.astype(idt), keep)


_CACHED_NC = None


def kernel(l_embeds: np.ndarray, l_labels: np.ndarray):
    global _CACHED_NC
    if _CACHED_NC is None:
        _CACHED_NC = build_program()
    nc = _CACHED_NC
    in_maps, aux = prepare(l_embeds, l_labels)
    res = run_bass_kernel_spmd(nc, in_maps, list(range(NCORES))).results
    return postprocess(res, aux)
